# revision 55
# baseline (speedup 1.0000x reference)
"""Trainium2 Bass kernel for nn_Detector (YOLO-style detector decode).

Contract: kernel(**inputs) takes the FULL unsharded inputs from
setup_inputs() and returns the FULL [340704, 90] fp32 output. The batch
dim (32) is sharded across 8 NeuronCores (4 images per core).

Design (v4, fp16 I/O, comp-major, no PE/PSUM):
  The decode is pure elementwise work, so the kernel is DMA-bound. The
  host pre-transposes each image into chunk layout (hw = c*128 +
  partition) and ships fp16, halving HBM bytes; the device does only
  decode math -- no TensorEngine, no PSUM.

  On-device tensors are COMP-MAJOR [128, comp, g] (g = 90 chunk-anchor
  groups innermost). The mask/scale broadcasts then sit on the outer
  free dim with a step-1 fp16 inner dim, which is what the DVE packed
  2x mode requires (a stride-0 inner dim drops to 1 elem/cycle).

  Precision (validated on the real data: fro ~3e-4, elementwise rel max
  ~2e-3, zero mask flips):
  - p/dx/dy stay f32 in a small side tensor: exact threshold compare on
    raw p vs logit(thresh); no (ix+dx) cancellation in fp16. dx/dy and
    the grid tables are pre-scaled by the stride t (a power of 2, exact).
  - point/seg-coord channels are pre-scaled x256 so neither fp16 input
    nor fp16 output hits the denormal range; the host divides those
    output columns by 256 after upcasting.
  - seg sigmoids use AF.Sigmoid directly (a tanh+affine form would
    round tanh~-1 through fp16 and amplify into small sigmoid outputs).

  ScalarE program order is pinned with explicit deps so the activation
  table-set rotation is [sigmoid] -> [exp] -> [sqrt] once per image
  pair (6 ACT_TABLE_LOADs total; the scheduler otherwise shuffles ACT
  ops and doubles the loads). Outputs are split into two DRAM tensors
  so the masked sigmoid block stores as soon as it is ready, before the
  sqrt/scale chain finishes the coord block.
"""
import numpy as np

f32np = np.float32
f16np = np.float16

B = 32
N_CORES = 8
B_LOCAL = B // N_CORES

# g-groups are scale-major: hw = c*128 + p, g = goff + c*3 + a
# (name, W, t, HW, T, goff)
SCALES = [("52", 52, 8.0, 2704, 22, 0),
          ("26", 26, 16.0, 676, 6, 66),
          ("13", 13, 32.0, 169, 2, 84)]
G = 90          # total groups = 3*(22+6+2)
NCOMP_H = 86    # fp16 comps: dw,dh | point*12 (x256) | segc*24 (x256) | segl*48
NCOMP_F = 3     # f32 comps: p, t*dx, t*dy
N_REST = 42     # out block 1: n, sig, cx, cy, w, h, point*12, segc*24
N_SIG = 48      # out block 2: seg logits -> sigmoids
SC = 256.0      # denormal-avoidance pre-scale on point/seg-coord channels

# consts column layout [128, NC] f32
_THRL = 0                 # 1 col: logit(thresh)
_NTAB = 1                 # 4 cols: n per local image
_AWF = 5                  # 180 cols: (aw,ah) as [2, 90] comp-major table
_IXYT = 185               # 180 cols: (t*ix, t*iy) as [2, 90] table
NC = 365

_CACHE = {}


def _build_nc(case):
    import concourse.bacc as bacc
    import concourse.tile as tile
    from concourse import mybir
    from concourse.tile_rust import add_dep_helper

    f32 = mybir.dt.float32
    f16 = mybir.dt.float16
    AF = mybir.ActivationFunctionType
    OP = mybir.AluOpType

    nc = bacc.Bacc("TRN2", target_bir_lowering=False, debug=False)
    # input split: the seg-logit block gates the long ScalarE sigmoid
    # chain, so it loads first per image; the rest (dw/dh/point/segc)
    # is only needed later (exp, then the post-sqrt coord scaling)
    xg = nc.declare_dram_parameter("xg", [B_LOCAL, 128, N_SIG, G], f16,
                                   isOutput=False)
    xr = nc.declare_dram_parameter("xr", [B_LOCAL, 128, 38, G], f16,
                                   isOutput=False)
    xf = nc.declare_dram_parameter("xf", [128, B_LOCAL, NCOMP_F, G], f32,
                                   isOutput=False)
    consts = nc.declare_dram_parameter("consts", [128, NC], f32,
                                       isOutput=False)
    y1 = nc.declare_dram_parameter("y1", [B_LOCAL, 128, N_REST, G], f16,
                                   isOutput=True)
    y2 = nc.declare_dram_parameter("y2", [B_LOCAL, 128, N_SIG, G], f16,
                                   isOutput=True)

    last_act = [None]

    def act(*args, **kw):
        # pin ScalarE program order so the table-set rotation holds
        ins = nc.scalar.activation(*args, **kw)
        if last_act[0] is not None:
            add_dep_helper(ins.ins, last_act[0].ins, sync=True,
                           reason="act table-set order")
        last_act[0] = ins
        return ins

    with tile.TileContext(nc) as tc:
        with (
            tc.tile_pool(name="single", bufs=1) as single,
            tc.tile_pool(name="inp", bufs=4) as in_pool,
            tc.tile_pool(name="outp", bufs=4) as out_pool,
            tc.tile_pool(name="small", bufs=4) as small,
        ):
            state = {}

            def load_g(b, split=1):
                in_g = in_pool.tile([128, N_SIG, G], f16, tag="ing")
                step = N_SIG // split
                for i in range(split):
                    # split the first load so the ScalarE sigmoid chain
                    # starts as soon as the first slice lands
                    nc.sync.dma_start(
                        out=in_g[:, i * step:(i + 1) * step, :],
                        in_=xg[b][:, i * step:(i + 1) * step, :])
                state.setdefault(b, {})["in_g"] = in_g

            def load_r(b):
                in_r = in_pool.tile([128, 38, G], f16, tag="inr")
                nc.sync.dma_start(out=in_r[:], in_=xr[b])
                state.setdefault(b, {})["in_r"] = in_r

            # ACT-critical seg-logit blocks lead; consts/xf ride behind
            load_g(0, split=4)
            load_g(1, split=2)
            ct = single.tile([128, NC], f32)
            nc.sync.dma_start(out=ct[:], in_=consts[:])
            xft = single.tile([128, B_LOCAL, NCOMP_F, G], f32)
            nc.sync.dma_start(out=xft[:], in_=xf[:])
            load_r(0)
            load_r(1)
            load_g(2)
            load_g(3)
            load_r(2)
            load_r(3)

            awf = ct[:, _AWF:_AWF + 180].rearrange("p (k g) -> p k g", g=G)
            ixyt = ct[:, _IXYT:_IXYT + 180].rearrange("p (k g) -> p k g", g=G)

            def phase_sg_all():
                # [sigmoid set] all four objectness sigmoids in one ACT
                # op (pre-masked p=-100 rows give exactly 0), then DVE
                # copies each image's row into its out tile
                sg_all = small.tile([128, B_LOCAL, G], f32, tag="sgall")
                act(sg_all[:], xft[:, :, 0, :], AF.Sigmoid)
                for b in range(B_LOCAL):
                    o_rest = out_pool.tile([128, N_REST, G], f16,
                                           tag="orest")
                    nc.vector.tensor_copy(o_rest[:, 1, :], sg_all[:, b, :])
                    state[b]["o_rest"] = o_rest

            def phase_segs(b, split=1):
                # [sigmoid set] seg sigmoids, full value, fp16 out
                o_sig = out_pool.tile([128, N_SIG, G], f16, tag="osig")
                in_g = state[b]["in_g"]
                step = N_SIG // split
                for i in range(split):
                    act(o_sig[:, i * step:(i + 1) * step, :],
                        in_g[:, i * step:(i + 1) * step, :], AF.Sigmoid)
                state[b]["o_sig"] = o_sig

            def phase_segs_streamed(b):
                # final images: sigmoid halves with the y2 store of each
                # half issued as soon as it is ready, so the tail drain
                # overlaps the remaining ACT work
                o_sig = out_pool.tile([128, N_SIG, G], f16, tag="osig")
                in_g = state[b]["in_g"]
                for i in (0, 1):
                    sl = slice(i * 24, (i + 1) * 24)
                    act(o_sig[:, sl, :], in_g[:, sl, :], AF.Sigmoid)
                    nc.sync.dma_start(out=y2[b][:, sl, :],
                                      in_=o_sig[:, sl, :])
                state[b]["o_sig"] = o_sig

            def store_y2(b):
                nc.sync.dma_start(out=y2[b], in_=state[b]["o_sig"][:])

            def phase_exp(b):
                # [exp set] wh = exp(dw,dh)
                wh = small.tile([128, 2, G], f32, tag="wh")
                act(wh[:], state[b]["in_r"][:, 0:2, :], AF.Exp)
                state[b]["wh"] = wh

            def phase_dve(b):
                # inputs are host pre-masked: masked rows have p=-100,
                # dw/dh=-30, seg logits=-20, point/segc=0, t*dx=-t*ix --
                # so every output lands at exactly 0 (or <fp16 denormal)
                # without any mask multiplies. The y2 sigmoid block goes
                # straight from ACT to DRAM.
                st = state[b]
                o_rest, wh = st["o_rest"], st["wh"]
                m = small.tile([128, G], f32, tag="m")
                nc.vector.tensor_scalar(m[:], xft[:, b, 0, :],
                                        ct[:, _THRL:_THRL + 1], None,
                                        op0=OP.is_gt)
                # row 0: n*m  (row 1 was written by ACT directly)
                nc.vector.tensor_scalar(
                    o_rest[:, 0, :], m[:], ct[:, _NTAB + b:_NTAB + b + 1],
                    None, op0=OP.mult)
                # rows 4,5: w,h = anchors * exp (in place on wh, f32)
                nc.vector.tensor_mul(wh[:], wh[:], awf)
                sq = small.tile([128, 2, G], f32, tag="sq")
                nc.vector.tensor_mul(sq[:], wh[:], wh[:])
                q = small.tile([128, G], f32, tag="q")
                nc.vector.tensor_add(q[:], sq[:, 0, :], sq[:, 1, :])
                nc.vector.tensor_copy(o_rest[:, 4:6, :], wh[:])
                # rows 2,3: t*dx + t*ix (masked rows cancel to exactly 0)
                nc.vector.tensor_add(o_rest[:, 2:4, :], xft[:, b, 1:3, :],
                                     ixyt)
                st["q"] = q

            def phase_s(b):
                # [sqrt set] s = sqrt(q)/case, batched per pair
                s = small.tile([128, G], f32, tag="s")
                act(s[:], state[b]["q"][:], AF.Sqrt,
                    scale=1.0 / (float(case) * float(case)))
                state[b]["s"] = s

            def phase_d(b):
                st = state[b]
                in_r, o_rest, s = st["in_r"], st["o_rest"], st["s"]
                s16 = small.tile([128, G], f16, tag="s16")
                nc.vector.tensor_copy(s16[:], s[:])
                # point + seg coords: fp16 x fp16 packed 2x
                nc.vector.tensor_mul(
                    o_rest[:, 6:42, :], in_r[:, 2:38, :],
                    s16[:].unsqueeze(1).broadcast_to((128, 36, G)))
                nc.gpsimd.dma_start(out=y1[b], in_=o_rest[:])

            # per-pair set rotation [sigmoid] -> [exp] -> [sqrt]; sg
            # ride the sigmoid set after segs1
            phase_segs(0, split=4)
            phase_segs(1, split=2)
            phase_sg_all()
            store_y2(0)
            store_y2(1)
            for b in (0, 1):
                phase_exp(b)         # [exp]
            for b in (0, 1):
                phase_dve(b)
            for b in (0, 1):
                phase_s(b)           # [sqrt]
            for b in (0, 1):
                phase_d(b)           # + y1 store
            phase_segs(2)            # [sigmoid]
            phase_segs(3)
            store_y2(2)
            store_y2(3)
            for b in (2, 3):
                phase_exp(b)         # [exp]
            for b in (2, 3):
                phase_dve(b)
            for b in (2, 3):
                phase_s(b)           # [sqrt]
            for b in (2, 3):
                phase_d(b)           # + y1 store
    nc.compile()
    return nc


# fp16 channel selection: index in original 90-comp input vector, in the
# device order [dw, dh, point*12, segcoord*24, seglogit*48]
_CHI = ([3, 4] + list(range(6, 18)) + list(range(18, 90, 3))
        + [c for k in range(24) for c in (19 + 3 * k, 20 + 3 * k)])
_CSCL = np.ones(NCOMP_H, f32np)
_CSCL[2:14] = SC   # point
_CSCL[14:38] = SC  # seg coords

# host unpack: final output column <- device comp index (y1 ++ y2)
_SRC = np.empty(90, np.int64)
_SRC[0:6] = np.arange(0, 6)
_SRC[6:18] = np.arange(6, 18)
_SRC[18:90:3] = 18 + np.arange(24)
_SRC[19:90:3] = 42 + 2 * np.arange(24)
_SRC[20:90:3] = 43 + 2 * np.arange(24)


def _host_consts(core, anchors, thr_logit):
    ct = np.zeros((128, NC), f32np)
    ct[:, _THRL] = thr_logit
    for b in range(B_LOCAL):
        ct[:, _NTAB + b] = f32np(core * B_LOCAL + b)
    awf = np.empty((2, G), f32np)
    ixyt = np.empty((2, G), f32np)
    for name, W, t, HW, T, goff in SCALES:
        a = anchors[name].astype(f32np)  # [3, 2] = (aw, ah) per anchor
        # g = goff + c*3 + anchor
        awf[0, goff:goff + 3 * T] = np.tile(a[:, 0], T)
        awf[1, goff:goff + 3 * T] = np.tile(a[:, 1], T)
        hw = np.arange(T) * 128  # chunk base; ix/iy vary per partition
        # per-partition values: fill later (they depend on p)
    # ixyt depends on the partition -> build full [128, 2, G]
    ixyt_f = np.empty((128, 2, G), f32np)
    for name, W, t, HW, T, goff in SCALES:
        hw = np.arange(T)[None, :] * 128 + np.arange(128)[:, None]  # [128,T]
        ix = (hw % W).astype(f32np) * f32np(t)
        iy = (hw // W).astype(f32np) * f32np(t)
        ixyt_f[:, 0, goff:goff + 3 * T] = np.repeat(ix, 3, axis=1)
        ixyt_f[:, 1, goff:goff + 3 * T] = np.repeat(iy, 3, axis=1)
    ct[:, _AWF:_AWF + 180] = np.broadcast_to(
        awf.reshape(-1)[None, :], (128, 180))
    ct[:, _IXYT:_IXYT + 180] = ixyt_f.reshape(128, 180)
    return ct


def _pack_inputs(out13, out26, out52, anchors, thresh):
    xs = {"13": np.asarray(out13, f32np), "26": np.asarray(out26, f32np),
          "52": np.asarray(out52, f32np)}
    thr = np.float64(np.asarray(thresh, f32np)[0])
    thr_logit = f32np(np.log(thr / (1.0 - thr)))

    xh = np.empty((B, 128, NCOMP_H, G), f16np)  # [dw,dh,point,segc | segl]
    xf_b = np.empty((B, 128, NCOMP_F, G), f32np)
    for name, W, t, HW, T, goff in SCALES:
        v = xs[name].reshape(B, 3, 90, HW)
        arr = np.zeros((B, 3, 90, T * 128), f32np)
        arr[..., :HW] = v
        # [B, 3, 90, T, 128] -> [B, 128, 90, T, 3] -> [B, 128, 90, 3T]
        arr = arr.reshape(B, 3, 90, T, 128).transpose(0, 4, 2, 3, 1)
        blk = np.ascontiguousarray(arr.reshape(B, 128, 90, 3 * T))
        # pre-mask: the device applies no mask multiplies; inputs are
        # conditioned so masked rows decode to exactly 0 (or below the
        # fp16 denormal cutoff). Mask = p > logit(thresh), the same f32
        # compare the device uses for the n column.
        pm = blk[:, :, 0, :] > thr_logit                  # [B, 128, 3T]
        pmn = ~pm
        blk[:, :, 0, :][pmn] = f32np(-100.0)              # p
        blk[:, :, 3, :][pmn] = f32np(-30.0)               # dw -> w ~ 1e-11
        blk[:, :, 4, :][pmn] = f32np(-30.0)               # dh
        for c in range(6, 18):
            blk[:, :, c, :][pmn] = f32np(0.0)             # point
        for c in range(18, 90, 3):
            blk[:, :, c, :][pmn] = f32np(0.0)             # seg coords
        for c in list(range(19, 90, 3)) + list(range(20, 90, 3)):
            blk[:, :, c, :][pmn] = f32np(-20.0)           # seg logits
        xh[:, :, :, goff:goff + 3 * T] = (
            blk[:, :, _CHI, :] * _CSCL[None, None, :, None]).astype(f16np)
        xf_b[:, :, 0, goff:goff + 3 * T] = blk[:, :, 0, :]
        # t*dx, with masked rows set to -t*ix so cx = t*dx + t*ix == 0
        hw = np.arange(T)[None, :] * 128 + np.arange(128)[:, None]
        ixg = np.repeat((hw % W).astype(f32np) * f32np(t), 3, axis=1)
        iyg = np.repeat((hw // W).astype(f32np) * f32np(t), 3, axis=1)
        tdx = blk[:, :, 1, :] * f32np(t)
        tdy = blk[:, :, 2, :] * f32np(t)
        tdx[pmn] = np.broadcast_to(-ixg[None], pmn.shape)[pmn]
        tdy[pmn] = np.broadcast_to(-iyg[None], pmn.shape)[pmn]
        xf_b[:, :, 1, goff:goff + 3 * T] = tdx
        xf_b[:, :, 2, goff:goff + 3 * T] = tdy
    in_maps = []
    for core in range(N_CORES):
        bs = slice(core * B_LOCAL, (core + 1) * B_LOCAL)
        m = {
            "xr": np.ascontiguousarray(xh[bs, :, 0:38]),
            "xg": np.ascontiguousarray(xh[bs, :, 38:86]),
            # [B_LOCAL,128,3,G] -> [128,B_LOCAL,3,G]
            "xf": np.ascontiguousarray(xf_b[bs].transpose(1, 0, 2, 3)),
            "consts": _host_consts(core, anchors, thr_logit),
        }
        in_maps.append(m)
    return in_maps


def _unpack_outputs(res):
    rows = {name: B * HW * 3 for name, _, _, HW, _, _ in SCALES}
    out = np.empty((rows["13"] + rows["26"] + rows["52"], 90), f32np)
    region = {"13": 0, "26": rows["13"], "52": rows["13"] + rows["26"]}
    for core in range(N_CORES):
        # [B_LOCAL, 128, 90, G] fp16, device comp order
        yv = np.concatenate([res[core]["y1"], res[core]["y2"]], axis=2)
        for name, W, t, HW, T, goff in SCALES:
            # [B_LOCAL, 128, 90, T, 3] -> [B_LOCAL, T, 128, 3, 90]
            arr = yv[:, :, :, goff:goff + 3 * T].reshape(
                B_LOCAL, 128, 90, T, 3).transpose(0, 3, 1, 4, 2)
            arr = arr.reshape(B_LOCAL, T * 128, 3, 90)[:, :HW]
            n = B_LOCAL * HW * 3
            out[region[name] + core * n:region[name] + (core + 1) * n] = \
                arr.reshape(n, 90)[:, _SRC].astype(f32np)
    out[:, 6:18] *= f32np(1.0 / SC)
    out[:, 18:90:3] *= f32np(1.0 / SC)
    return out


def kernel(out13, out26, out52, anchors13, anchors26, anchors52, thresh,
           case, **kw):
    from concourse.bass_utils import run_bass_kernel_spmd

    anchors = {"13": np.asarray(anchors13), "26": np.asarray(anchors26),
               "52": np.asarray(anchors52)}
    key = ("nc", int(case))
    if key not in _CACHE:
        _CACHE[key] = _build_nc(int(case))
    nc = _CACHE[key]

    in_maps = _pack_inputs(out13, out26, out52, anchors,
                           np.asarray(thresh, f32np))
    res = run_bass_kernel_spmd(nc, in_maps, list(range(N_CORES))).results
    return _unpack_outputs(res)
